# revision 4
# baseline (speedup 1.0000x reference)
"""Grouped-linear (EvolvedLoopLinear) Trainium2 Bass kernel.

Problem: out[b, j] = sum_s x[b, g*64+s] * weight[j, g*64+s] + bias[j],
with g = j % 128, for x [4096, 8192], weight [4096, 8192], bias [4096].

Strategy: data-parallel over batch across 8 cores (512 rows each).
Per core:
  - x arrives batch-on-partitions; the contraction dim must be on partitions
    for the PE, so [128,128] tiles are PE-transposed into per-group-pair
    "xT" slabs (rounded to float32r by the PSUM->SBUF evacuation copy).
  - Matmuls use the (host-prepared) block-diagonal gathered weight pairs as
    the stationary operand in float32r (full-rate fp32 path, N=256).
    Output lands transposed (j on partitions); pairs are packed 4-to-a-bank.
  - PE back-transposes restore batch-on-partitions, and a DVE scatter-copy
    with fused bias-add writes the interleaved j columns (j = m*128 + g)
    into a contiguous out tile, which is stored with plain 2MB DMAs.

Host-side prep is limited to the small parameter tensors: the gathered
block-diagonal weight pairs (1MB of live weight data), a replicated bias,
and a 128x128 identity for the PE transposes.
"""
import numpy as np
from contextlib import ExitStack

import concourse.bass as bass
import concourse.tile as tile
import concourse.tile_sem_assignment as _tsa
from concourse import bacc, mybir
from concourse.bass_utils import run_bass_kernel_spmd

# The walrus build in this container rejects instructions carrying more than
# a couple of semaphore waits ("Too many sync wait commands"); capping the
# HWDGE completion lanes keeps the kernel-tail drain under that limit.
_tsa.NUM_HWDGE_SEMS = 2

BATCH = 4096
IN_F = 8192
OUT_F = 4096
GROUPS = 128
STEP = 64
M_PER_G = 32          # outputs per group
N_CORES = 8
B_CORE = BATCH // N_CORES      # 512
N_PAIR = GROUPS // 2           # 64 group pairs
HALF_B = B_CORE // 2           # 256 batch rows per half
SLAB_COLS = 2048               # x load slab width (16 pairs)

f32 = mybir.dt.float32
f32r = mybir.dt.float32r

_COMPILED = {}


def _build():
    if "nc" in _COMPILED:
        return _COMPILED["nc"]

    nc = bacc.Bacc("TRN2", target_bir_lowering=False, debug=False)
    x_ap = nc.dram_tensor("x_s", [B_CORE, IN_F], f32, kind="ExternalInput").ap()
    w_ap = nc.dram_tensor("w_bd", [128, N_PAIR * 64], f32r, kind="ExternalInput").ap()
    b_ap = nc.dram_tensor("bias_rep", [128, OUT_F], f32, kind="ExternalInput").ap()
    i_ap = nc.dram_tensor("ident", [128, 128], f32, kind="ExternalInput").ap()
    y_ap = nc.dram_tensor("out_s", [B_CORE, OUT_F], f32, kind="ExternalOutput").ap()

    with tile.TileContext(nc) as tc:
        with ExitStack() as ctx:
            const_pool = ctx.enter_context(tc.tile_pool(name="const", bufs=1))
            slab_pool = ctx.enter_context(tc.tile_pool(name="slab", bufs=4))
            xt_pool = ctx.enter_context(tc.tile_pool(name="xt", bufs=3))
            ot_pool = ctx.enter_context(tc.tile_pool(name="ot", bufs=3))
            osb_pool = ctx.enter_context(tc.tile_pool(name="osb", bufs=3))
            psA_pool = ctx.enter_context(tc.tile_pool(name="psA", bufs=2, space="PSUM"))
            psB_pool = ctx.enter_context(tc.tile_pool(name="psB", bufs=2, space="PSUM"))
            psC_pool = ctx.enter_context(tc.tile_pool(name="psC", bufs=2, space="PSUM"))

            w_sb = const_pool.tile([128, N_PAIR * 64], f32r)
            nc.sync.dma_start(w_sb[:], w_ap[:])
            bias_sb = const_pool.tile([128, OUT_F], f32)
            nc.sync.dma_start(bias_sb[:], b_ap[:])
            ident = const_pool.tile([128, 128], f32)
            nc.sync.dma_start(ident[:], i_ap[:])

            n_slab = IN_F // SLAB_COLS          # 4 column slabs
            quads_per_slab = SLAB_COLS // 512   # 4 quads per slab

            for half in range(2):
                b0 = half * HALF_B
                slabs = [[None] * n_slab for _ in range(2)]
                out_sb = [osb_pool.tile([128, OUT_F], f32, tag="osb", name=f"osb_{half}_{i}")
                          for i in range(2)]
                psC = [None, None]

                for q in range(16):  # quads of 4 pairs
                    cs = q // quads_per_slab
                    if q % quads_per_slab == 0:
                        for bt2 in range(2):
                            s = slab_pool.tile([128, SLAB_COLS], f32, tag="slab")
                            nc.sync.dma_start(
                                s[:],
                                x_ap[b0 + bt2 * 128:b0 + bt2 * 128 + 128,
                                     cs * SLAB_COLS:(cs + 1) * SLAB_COLS])
                            slabs[bt2][cs] = s
                    qq = q % 2
                    if qq == 0:
                        psC = [psC_pool.tile([128, 512], f32, tag="psC", name=f"psC_{half}_{q}_{i}")
                               for i in range(2)]

                    # --- xT production: 2 duos per quad ---
                    xt_duo = []
                    for dk in range(2):
                        psA = psA_pool.tile([128, 512], f32, tag="psA")
                        for pk in range(2):
                            for bt2 in range(2):
                                k = 4 * q + 2 * dk + pk
                                coff = k * 128 - cs * SLAB_COLS
                                nc.tensor.transpose(
                                    psA[:, (pk * 2 + bt2) * 128:(pk * 2 + bt2) * 128 + 128],
                                    slabs[bt2][cs][:, coff:coff + 128],
                                    ident[:])
                        xt = xt_pool.tile([128, 512], f32r, tag="xt")
                        nc.vector.tensor_copy(xt[:], psA[:])
                        xt_duo.append(xt)

                    # --- matmuls: 4 pairs along columns of a 2-bank psum tile
                    #     (f32r matmul output must start at partition 0) ---
                    psB = psB_pool.tile([64, 1024], f32, tag="psB")
                    for ka in range(4):      # kappa: pair within quad
                        k = 4 * q + ka
                        nc.tensor.matmul(
                            psB[:, ka * 256:ka * 256 + 256],
                            w_sb[:, k * 64:(k + 1) * 64],
                            xt_duo[ka // 2][:, (ka % 2) * 256:(ka % 2) * 256 + 256],
                            start=True, stop=True)

                    # --- evacuate out^T and back-transpose ---
                    ot = ot_pool.tile([64, 1024], f32, tag="ot")
                    nc.scalar.copy(ot[:], psB[:])
                    for ka in range(4):      # kappa
                        for bt2 in range(2):
                            nc.tensor.transpose(
                                psC[bt2][:, (qq * 4 + ka) * 64:(qq * 4 + ka) * 64 + 64],
                                ot[:, ka * 256 + bt2 * 128:ka * 256 + bt2 * 128 + 128],
                                ident[:64, :64])

                    # --- scatter-evacuate with bias add every 2 quads ---
                    if qq == 1:
                        t = q // 2
                        for bt2 in range(2):
                            src = psC[bt2][:].rearrange("p (i m) -> p i m", i=16)
                            dst = out_sb[bt2][:].rearrange(
                                "p (m i) -> p i m", m=M_PER_G)[:, 16 * t:16 * t + 16, :]
                            bsl = bias_sb[:].rearrange(
                                "p (m i) -> p i m", m=M_PER_G)[:, 16 * t:16 * t + 16, :]
                            nc.vector.tensor_tensor(
                                dst, src, bsl, op=mybir.AluOpType.add)

                for bt2 in range(2):
                    nc.sync.dma_start(
                        y_ap[b0 + bt2 * 128:b0 + bt2 * 128 + 128, :],
                        out_sb[bt2][:])

    nc.compile()
    _COMPILED["nc"] = nc
    return nc


def _host_prep(weight, bias):
    # gather: Wg[j, s] = weight[j, (j%128)*64 + s]
    j = np.arange(OUT_F)
    Wg = weight.reshape(OUT_F, GROUPS, STEP)[j, j % GROUPS]          # [4096, 64]
    W4 = Wg.reshape(M_PER_G, GROUPS, STEP)                           # [m, g, s]
    Wk = W4.reshape(M_PER_G, N_PAIR, 2, STEP)                        # [m, k, h, s]
    # block-diagonal pair stationary: w_bd[64h + s, 64k + 32h' + m]
    w_bd = np.zeros((2, STEP, N_PAIR, 2, M_PER_G), dtype=np.float32)
    for h in range(2):
        w_bd[h, :, :, h, :] = Wk[:, :, h, :].transpose(2, 1, 0)      # [s, k, m]
    w_bd = np.ascontiguousarray(w_bd.reshape(128, N_PAIR * 64))

    bias_rep = np.ascontiguousarray(
        np.broadcast_to(bias.astype(np.float32), (128, OUT_F)))
    ident = np.eye(128, dtype=np.float32)
    return w_bd, bias_rep, ident


def kernel(x, weight, bias):
    x = np.asarray(x, dtype=np.float32)
    weight = np.asarray(weight, dtype=np.float32)
    bias = np.asarray(bias, dtype=np.float32)

    nc = _build()
    w_bd, bias_rep, ident = _host_prep(weight, bias)

    in_maps = []
    for c in range(N_CORES):
        in_maps.append({
            "x_s": np.ascontiguousarray(x[c * B_CORE:(c + 1) * B_CORE]),
            "w_bd": w_bd,
            "bias_rep": bias_rep,
            "ident": ident,
        })
    res = run_bass_kernel_spmd(nc, in_maps, core_ids=list(range(N_CORES)))
    out = np.concatenate([res.results[c]["out_s"] for c in range(N_CORES)], axis=0)
    return out
